# revision 10
# baseline (speedup 1.0000x reference)
"""Trainium2 Bass kernel for a SwiGLU-style feed-forward block.

reference:
    gate = x @ w1.T ; up = x @ w2.T ; h = silu(gate) * up ; out = h @ w3.T
    x: [4, 2048, 2048] f32, w1/w2: [8192, 2048] f32, w3: [2048, 8192] f32

Strategy: pure data-parallel over the 8192 tokens - each of the 8
NeuronCores gets 1024 tokens and the full weights.  All 1024 tokens
stay resident in SBUF so every weight tile is streamed from HBM
exactly ONCE (~105MB/core instead of ~220MB), halving DMA energy and
the power-throttle pressure.  Both 512-token halves are computed
back-to-back per weight tile, sharing the PE's stationary operand.

All tensors are pre-TILED + cast to bf16 on the host so every DMA is
a contiguous 4-16KB-per-partition read:

    xc0/xc1 [128, 4(q), 4(s), 512]      activations, 4 parts per half
    w1h0/w2h0 [128, 16(s), 128]         first slab, split for startup
    w12   [128, 63(hb), 16(s), 256]     merged w1|w2 slabs (128+128)
    w3t   [128, 16(et), 2, 32(hs), 128]

phase A (per 256-col slab): gateT/upT = w-tile^T @ x-tile (PSUM)
         hT = silu(gateT) * upT  (ACT Silu + DVE mul, bf16)
phase B: outT[e,:] = sum_h w3-tile^T @ hT   (PSUM accum over H)
Output is outT [E, 1024] bf16 per core; the host reassembles.
"""

import json

import numpy as np
import ml_dtypes

import concourse.bass as bass
import concourse.mybir as mybir
import concourse.tile as tile
from concourse.vector_clock import ScopedClock
from concourse.bass_utils import run_bass_kernel_spmd

# ---------------------------------------------------------------- shapes
N_CORES = 8
EMB = 2048          # E
HID = 8192          # H
T_TOTAL = 8192      # B*S tokens
T_SHARD = T_TOTAL // N_CORES   # 1024 tokens per core
T_CHUNK = 512                  # tokens per PSUM group
E_SUB = EMB // 128             # 16 contraction subtiles for phase A
H_SUB = HID // 128             # 64 contraction subtiles for phase B
N_SLAB = HID // 128            # 64 slabs: 128 h-cols of w1 | same 128 of w2

CDT = mybir.dt.bfloat16        # compute dtype on the PE
NP_CDT = ml_dtypes.bfloat16

P = 128
F32 = mybir.dt.float32


class _TileContextSplitWait(tile.TileContext):
    """The walrus build in this environment rejects >1 sync-wait on a
    CTRL (Drain) instruction.  Split the kernel-tail drain's waits into
    single-wait nops emitted just before it."""

    def _drain_and_barrier(self, tick_clock, wait_clock):
        probe = self.nc.sync.nop(nofuse=True)
        wait_clock.add_sem_waits(
            probe.ins, ScopedClock({None: tick_clock.global_clock})
        )
        si = probe.ins.sync_info
        if si is not None and len(si.on_wait) > 1:
            waits = list(si.on_wait)
            probe.ins.sync_info = mybir.SyncInfo(
                on_wait=waits[:1], on_update=list(si.on_update)
            )
            for w in waits[1:]:
                n = self.nc.sync.nop(nofuse=True)
                n.ins.sync_info = mybir.SyncInfo(on_wait=[w], on_update=[])
        self.nc.sync.drain()
        self.nc.all_engine_barrier()
        assert self.sems is not None
        popped = self.nc._tile_sem_poison_stack.pop()
        assert popped is self._sem_poison
        self.nc.clear_and_free_semaphores(list(self.sems.allocated().values()))
        self.nc.all_engine_barrier()


def _split_multi_waits(bir_bytes):
    """The walrus build here accepts at most one sync-wait command per
    instruction (setupSyncWait raises 'Too many sync wait commands').
    Tile attaches however many the dependence analysis needs, so move
    extra waits onto NoOp instructions inserted just before, on the same
    engine's stream - semantically identical, codegen-compatible."""
    bir = json.loads(bir_bytes)
    for fn in bir["functions"]:
        for blk in fn["blocks"]:
            insts = blk.get("instructions")
            if not insts:
                continue
            out = []
            changed = False
            for inst in insts:
                si = inst.get("sync_info")
                waits = (si or {}).get("on_wait") or []
                if len(waits) > 1:
                    changed = True
                    for j, w in enumerate(waits[:-1]):
                        out.append(
                            {
                                "debug": inst.get("debug"),
                                "engine": inst["engine"],
                                "ins": [],
                                "name": f"{inst['name']}-w{j}",
                                "opcode": "NoOp",
                                "outs": [],
                                "sync_info": {"on_update": [], "on_wait": [w]},
                            }
                        )
                    si["on_wait"] = waits[-1:]
                out.append(inst)
            if changed:
                blk["instructions"] = out
    return json.dumps(bir).encode()


def _build_nc():
    nc = bass.Bass(target_bir_lowering=False)

    xc0 = nc.dram_tensor("xc0", [P, 4, 4, T_CHUNK], CDT, kind="ExternalInput")
    xc1 = nc.dram_tensor("xc1", [P, 4, 4, T_CHUNK], CDT, kind="ExternalInput")
    w1h0 = nc.dram_tensor("w1h0", [P, E_SUB, P], CDT, kind="ExternalInput")
    w2h0 = nc.dram_tensor("w2h0", [P, E_SUB, P], CDT, kind="ExternalInput")
    w12 = nc.dram_tensor(
        "w12", [P, N_SLAB - 1, E_SUB, 256], CDT, kind="ExternalInput"
    )
    w3t = nc.dram_tensor(
        "w3t", [P, E_SUB, 2, H_SUB // 2, P], CDT, kind="ExternalInput"
    )
    outt = nc.dram_tensor("outt", [EMB, T_SHARD], CDT, kind="ExternalOutput")

    xc0_r, xc1_r = xc0[:], xc1[:]
    w1h0_r, w2h0_r = w1h0[:], w2h0[:]
    w12_r, w3t_r = w12[:], w3t[:]

    with _TileContextSplitWait(nc) as tc:
        with (
            tc.tile_pool(name="xp", bufs=1) as xp,
            tc.tile_pool(name="qp", bufs=1) as qp,
            tc.tile_pool(name="wp", bufs=2) as wp,
            tc.tile_pool(name="w3p", bufs=1) as w3p,
            tc.tile_pool(name="htp", bufs=1) as htp,
            tc.tile_pool(name="slp", bufs=2) as slp,
            tc.tile_pool(name="op", bufs=2) as op,
            tc.tile_pool(name="ps", bufs=2, space="PSUM") as ps,
        ):
            # PE p-state warmup: dummy matmuls on a zeroed tile run at
            # the reduced ramp clock during the initial DMA-wait window,
            # so the first real matmuls start at full frequency.
            zt = xp.tile([P, T_CHUNK], CDT, name="zt")
            nc.vector.memset(zt[:], 0.0)
            wm = ps.tile([P, T_CHUNK], F32, name="pg")
            for _ in range(8):
                nc.tensor.matmul(
                    wm[:], zt[:, 0:P], zt[:], start=True, stop=True
                )

            # x for both halves, in 4 quarter tiles each
            xq = [
                xp.tile([P, 4, T_CHUNK], CDT, name=f"xs{q}") for q in range(8)
            ]
            w1q = qp.tile([P, E_SUB, P], CDT, name="w1h0")
            w2q = qp.tile([P, E_SUB, P], CDT, name="w2h0")
            nc.sync.dma_start(xq[0][:], xc0_r[:, 0])
            nc.sync.dma_start(w1q[:], w1h0_r)
            for q in range(1, 4):
                nc.sync.dma_start(xq[q][:], xc0_r[:, q])
            nc.sync.dma_start(w2q[:], w2h0_r)
            for q in range(4):
                nc.sync.dma_start(xq[4 + q][:], xc1_r[:, q])

            def xv(c, e):
                return xq[4 * c + e // 4][:, e % 4, :]

            ht = htp.tile([P, H_SUB, T_SHARD], CDT, name="ht")

            # ---------------- phase A: gate/up + silu*up -> hT
            def silu_mul(hsub, c, pg, pu):
                sl = slp.tile([P, T_CHUNK], CDT, name="sl")
                nc.scalar.activation(
                    sl[:], pg[:], mybir.ActivationFunctionType.Silu
                )
                nc.vector.tensor_mul(
                    ht[:, hsub, c * T_CHUNK : (c + 1) * T_CHUNK], sl[:], pu[:]
                )

            # slab 0: chunk-sequential so the first group only needs
            # xq0 + w1h0 (1MB) - minimizes time-to-first-matmul.
            for c in range(2):
                pg = ps.tile([P, T_CHUNK], F32, name="pg")
                for e in range(E_SUB):
                    nc.tensor.matmul(
                        pg[:], w1q[:, e, :], xv(c, e),
                        start=(e == 0), stop=(e == E_SUB - 1),
                    )
                pu = ps.tile([P, T_CHUNK], F32, name="pu")
                for e in range(E_SUB):
                    nc.tensor.matmul(
                        pu[:], w2q[:, e, :], xv(c, e),
                        start=(e == 0), stop=(e == E_SUB - 1),
                    )
                silu_mul(0, c, pg, pu)

            # slabs 1..63: both chunks back-to-back per weight column
            # block (shared stationary operand on the PE).
            for hb in range(1, N_SLAB):
                w12s = wp.tile([P, E_SUB, 256], CDT, name="w12s")
                nc.sync.dma_start(w12s[:], w12_r[:, hb - 1])

                pgs = [ps.tile([P, T_CHUNK], F32, name="pg") for _ in range(2)]
                for e in range(E_SUB):
                    for c in range(2):
                        nc.tensor.matmul(
                            pgs[c][:], w12s[:, e, 0:P], xv(c, e),
                            start=(e == 0), stop=(e == E_SUB - 1),
                        )
                pus = [ps.tile([P, T_CHUNK], F32, name="pu") for _ in range(2)]
                for e in range(E_SUB):
                    for c in range(2):
                        nc.tensor.matmul(
                            pus[c][:], w12s[:, e, P:256], xv(c, e),
                            start=(e == 0), stop=(e == E_SUB - 1),
                        )
                for c in range(2):
                    silu_mul(hb, c, pgs[c], pus[c])

            # ---------------- phase B: outT = sum_h w3T^T @ hT
            hh = H_SUB // 2

            def out_store(et, c, po):
                e0 = et * P
                ot = op.tile([P, T_CHUNK], CDT, name="ot")
                nc.vector.tensor_copy(ot[:], po[:])
                nc.sync.dma_start(
                    outt[e0 : e0 + P, c * T_CHUNK : (c + 1) * T_CHUNK], ot[:]
                )

            for et in range(E_SUB):
                w3a = w3p.tile([P, hh, P], CDT, name="w3a")
                nc.sync.dma_start(w3a[:], w3t_r[:, et, 0])
                w3b = w3p.tile([P, hh, P], CDT, name="w3b")
                nc.sync.dma_start(w3b[:], w3t_r[:, et, 1])

                def w3v(h):
                    return w3a[:, h, :] if h < hh else w3b[:, h - hh, :]

                if et < E_SUB - 1:
                    pos = [
                        ps.tile([P, T_CHUNK], F32, name="po", bufs=4)
                        for _ in range(2)
                    ]
                    for h in range(H_SUB):
                        for c in range(2):
                            nc.tensor.matmul(
                                pos[c][:], w3v(h),
                                ht[:, h, c * T_CHUNK : (c + 1) * T_CHUNK],
                                start=(h == 0), stop=(h == H_SUB - 1),
                            )
                    for c in range(2):
                        out_store(et, c, pos[c])
                else:
                    # last e-tile: chunk-sequential so chunk 0's copy +
                    # store DMA overlap chunk 1's matmuls (shorter tail)
                    for c in range(2):
                        po = ps.tile([P, T_CHUNK], F32, name="po", bufs=4)
                        for h in range(H_SUB):
                            nc.tensor.matmul(
                                po[:], w3v(h),
                                ht[:, h, c * T_CHUNK : (c + 1) * T_CHUNK],
                                start=(h == 0), stop=(h == H_SUB - 1),
                            )
                        out_store(et, c, po)

    fixed = _split_multi_waits(bass.Bass.to_json_bytes(nc))
    nc.to_json_bytes = lambda: fixed
    return nc


_nc_cache = None


def _get_nc():
    global _nc_cache
    if _nc_cache is None:
        _nc_cache = _build_nc()
    return _nc_cache


def _prep_inputs(x, w1, w2, w3):
    xt = np.ascontiguousarray(
        x.reshape(T_TOTAL, EMB).T.astype(NP_CDT)
    )  # [E, T_total]

    # merged w1|w2 slabs: [p][hb][s][256] with cols 0:128 = w1, 128:256 = w2
    a1 = w1.T.astype(NP_CDT).reshape(E_SUB, P, N_SLAB, P).transpose(1, 2, 0, 3)
    a2 = w2.T.astype(NP_CDT).reshape(E_SUB, P, N_SLAB, P).transpose(1, 2, 0, 3)
    w1h0 = np.ascontiguousarray(a1[:, 0])  # [128, 16, 128]
    w2h0 = np.ascontiguousarray(a2[:, 0])
    w12 = np.ascontiguousarray(
        np.concatenate([a1, a2], axis=3)[:, 1:]
    )  # [128, 63, 16, 256]

    # w3 tiles: [p][et][half][hs][ec]
    w3r = w3.T.astype(NP_CDT).reshape(H_SUB, P, E_SUB, P)
    w3tl = np.ascontiguousarray(
        w3r.transpose(1, 2, 0, 3).reshape(P, E_SUB, 2, H_SUB // 2, P)
    )

    in_maps = []
    for i in range(N_CORES):
        sh = xt[:, i * T_SHARD : (i + 1) * T_SHARD]  # [E, 1024]
        X = sh.reshape(E_SUB, P, T_SHARD)  # [s, p, t]
        xc0 = np.ascontiguousarray(
            X[:, :, :T_CHUNK].reshape(4, 4, P, T_CHUNK).transpose(2, 0, 1, 3)
        )  # [128, 4, 4, 512]
        xc1 = np.ascontiguousarray(
            X[:, :, T_CHUNK:].reshape(4, 4, P, T_CHUNK).transpose(2, 0, 1, 3)
        )
        in_maps.append(
            {
                "xc0": xc0,
                "xc1": xc1,
                "w1h0": w1h0,
                "w2h0": w2h0,
                "w12": w12,
                "w3t": w3tl,
            }
        )
    return in_maps


def kernel(x, w1, w2, w3, scale_x=None, _trace=False):
    x = np.asarray(x, np.float32)
    w1 = np.asarray(w1, np.float32)
    w2 = np.asarray(w2, np.float32)
    w3 = np.asarray(w3, np.float32)

    nc = _get_nc()
    in_maps = _prep_inputs(x, w1, w2, w3)
    res = run_bass_kernel_spmd(nc, in_maps, list(range(N_CORES)), trace=_trace)

    outt = np.concatenate(
        [
            np.asarray(res.results[i]["outt"]).astype(np.float32)
            for i in range(N_CORES)
        ],
        axis=1,
    )  # [E, T_total]
    out = np.ascontiguousarray(outt.T).reshape(4, 2048, EMB).astype(np.float32)
    if _trace:
        kernel.last_results = res
    return out


if __name__ == "__main__":
    rng = np.random.default_rng(0)
    x = rng.standard_normal((4, 2048, EMB), dtype=np.float32)
    w1 = (rng.standard_normal((HID, EMB), dtype=np.float32) * 0.03).astype(
        np.float32
    )
    w2 = (rng.standard_normal((HID, EMB), dtype=np.float32) * 0.03).astype(
        np.float32
    )
    w3 = (rng.standard_normal((EMB, HID), dtype=np.float32) * 0.015).astype(
        np.float32
    )
    out = kernel(x, w1, w2, w3)
    print("out", out.shape, out.dtype, float(np.abs(out).mean()))


# revision 11
# speedup vs baseline: 1.0010x; 1.0010x over previous
"""Trainium2 Bass kernel for a SwiGLU-style feed-forward block.

reference:
    gate = x @ w1.T ; up = x @ w2.T ; h = silu(gate) * up ; out = h @ w3.T
    x: [4, 2048, 2048] f32, w1/w2: [8192, 2048] f32, w3: [2048, 8192] f32

Strategy: pure data-parallel over the 8192 tokens - each of the 8
NeuronCores gets 1024 tokens and the full weights.  All 1024 tokens
stay resident in SBUF so every weight tile is streamed from HBM
exactly ONCE (~105MB/core instead of ~220MB), halving DMA energy and
the power-throttle pressure.  Both 512-token halves are computed
back-to-back per weight tile, sharing the PE's stationary operand.

All tensors are pre-TILED + cast to bf16 on the host so every DMA is
a contiguous 4-16KB-per-partition read:

    xc0/xc1 [128, 4(q), 4(s), 512]      activations, 4 parts per half
    w1h0/w2h0 [128, 16(s), 128]         first slab, split for startup
    w12   [128, 63(hb), 16(s), 256]     merged w1|w2 slabs (128+128)
    w3t   [128, 16(et), 2, 32(hs), 128]

phase A (per 256-col slab): gateT/upT = w-tile^T @ x-tile (PSUM)
         hT = silu(gateT) * upT  (ACT Silu + DVE mul, bf16)
phase B: outT[e,:] = sum_h w3-tile^T @ hT   (PSUM accum over H)
Output is outT [E, 1024] bf16 per core; the host reassembles.
"""

import json

import numpy as np
import ml_dtypes

import concourse.bass as bass
import concourse.mybir as mybir
import concourse.tile as tile
from concourse.vector_clock import ScopedClock
from concourse.bass_utils import run_bass_kernel_spmd

# ---------------------------------------------------------------- shapes
N_CORES = 8
EMB = 2048          # E
HID = 8192          # H
T_TOTAL = 8192      # B*S tokens
T_SHARD = T_TOTAL // N_CORES   # 1024 tokens per core
T_CHUNK = 512                  # tokens per PSUM group
E_SUB = EMB // 128             # 16 contraction subtiles for phase A
H_SUB = HID // 128             # 64 contraction subtiles for phase B
N_SLAB = HID // 128            # 64 slabs: 128 h-cols of w1 | same 128 of w2

CDT = mybir.dt.bfloat16        # compute dtype on the PE
NP_CDT = ml_dtypes.bfloat16

P = 128
F32 = mybir.dt.float32


class _TileContextSplitWait(tile.TileContext):
    """The walrus build in this environment rejects >1 sync-wait on a
    CTRL (Drain) instruction.  Split the kernel-tail drain's waits into
    single-wait nops emitted just before it."""

    def _drain_and_barrier(self, tick_clock, wait_clock):
        probe = self.nc.sync.nop(nofuse=True)
        wait_clock.add_sem_waits(
            probe.ins, ScopedClock({None: tick_clock.global_clock})
        )
        si = probe.ins.sync_info
        if si is not None and len(si.on_wait) > 1:
            waits = list(si.on_wait)
            probe.ins.sync_info = mybir.SyncInfo(
                on_wait=waits[:1], on_update=list(si.on_update)
            )
            for w in waits[1:]:
                n = self.nc.sync.nop(nofuse=True)
                n.ins.sync_info = mybir.SyncInfo(on_wait=[w], on_update=[])
        self.nc.sync.drain()
        self.nc.all_engine_barrier()
        assert self.sems is not None
        popped = self.nc._tile_sem_poison_stack.pop()
        assert popped is self._sem_poison
        self.nc.clear_and_free_semaphores(list(self.sems.allocated().values()))
        self.nc.all_engine_barrier()


def _split_multi_waits(bir_bytes):
    """The walrus build here accepts at most one sync-wait command per
    instruction (setupSyncWait raises 'Too many sync wait commands').
    Tile attaches however many the dependence analysis needs, so move
    extra waits onto NoOp instructions inserted just before, on the same
    engine's stream - semantically identical, codegen-compatible."""
    bir = json.loads(bir_bytes)
    for fn in bir["functions"]:
        for blk in fn["blocks"]:
            insts = blk.get("instructions")
            if not insts:
                continue
            out = []
            changed = False
            for inst in insts:
                si = inst.get("sync_info")
                waits = (si or {}).get("on_wait") or []
                if len(waits) > 1:
                    changed = True
                    for j, w in enumerate(waits[:-1]):
                        out.append(
                            {
                                "debug": inst.get("debug"),
                                "engine": inst["engine"],
                                "ins": [],
                                "name": f"{inst['name']}-w{j}",
                                "opcode": "NoOp",
                                "outs": [],
                                "sync_info": {"on_update": [], "on_wait": [w]},
                            }
                        )
                    si["on_wait"] = waits[-1:]
                out.append(inst)
            if changed:
                blk["instructions"] = out
    return json.dumps(bir).encode()


def _build_nc():
    nc = bass.Bass(target_bir_lowering=False)

    xc0 = nc.dram_tensor("xc0", [P, 4, 4, T_CHUNK], CDT, kind="ExternalInput")
    xc1 = nc.dram_tensor("xc1", [P, 4, 4, T_CHUNK], CDT, kind="ExternalInput")
    w1h0 = nc.dram_tensor("w1h0", [P, E_SUB, P], CDT, kind="ExternalInput")
    w2h0 = nc.dram_tensor("w2h0", [P, E_SUB, P], CDT, kind="ExternalInput")
    w12 = nc.dram_tensor(
        "w12", [P, N_SLAB - 1, E_SUB, 256], CDT, kind="ExternalInput"
    )
    w3t = nc.dram_tensor(
        "w3t", [P, E_SUB, 2, H_SUB // 2, P], CDT, kind="ExternalInput"
    )
    outt = nc.dram_tensor("outt", [EMB, T_SHARD], CDT, kind="ExternalOutput")

    xc0_r, xc1_r = xc0[:], xc1[:]
    w1h0_r, w2h0_r = w1h0[:], w2h0[:]
    w12_r, w3t_r = w12[:], w3t[:]

    with _TileContextSplitWait(nc) as tc:
        with (
            tc.tile_pool(name="xp", bufs=1) as xp,
            tc.tile_pool(name="qp", bufs=1) as qp,
            tc.tile_pool(name="wp", bufs=2) as wp,
            tc.tile_pool(name="w3p", bufs=1) as w3p,
            tc.tile_pool(name="htp", bufs=1) as htp,
            tc.tile_pool(name="slp", bufs=2) as slp,
            tc.tile_pool(name="op", bufs=2) as op,
            tc.tile_pool(name="ps", bufs=2, space="PSUM") as ps,
        ):
            # x for both halves, in 4 quarter tiles each
            xq = [
                xp.tile([P, 4, T_CHUNK], CDT, name=f"xs{q}") for q in range(8)
            ]
            w1q = qp.tile([P, E_SUB, P], CDT, name="w1h0")
            w2q = qp.tile([P, E_SUB, P], CDT, name="w2h0")
            nc.sync.dma_start(xq[0][:], xc0_r[:, 0])
            nc.sync.dma_start(w1q[:], w1h0_r)
            for q in range(1, 4):
                nc.sync.dma_start(xq[q][:], xc0_r[:, q])
            nc.sync.dma_start(w2q[:], w2h0_r)
            for q in range(4):
                nc.sync.dma_start(xq[4 + q][:], xc1_r[:, q])

            def xv(c, e):
                return xq[4 * c + e // 4][:, e % 4, :]

            ht = htp.tile([P, H_SUB, T_SHARD], CDT, name="ht")

            # ---------------- phase A: gate/up + silu*up -> hT
            def silu_mul(hsub, c, pg, pu):
                sl = slp.tile([P, T_CHUNK], CDT, name="sl")
                nc.scalar.activation(
                    sl[:], pg[:], mybir.ActivationFunctionType.Silu
                )
                nc.vector.tensor_mul(
                    ht[:, hsub, c * T_CHUNK : (c + 1) * T_CHUNK], sl[:], pu[:]
                )

            # slab 0: chunk-sequential so the first group only needs
            # xq0 + w1h0 (1MB) - minimizes time-to-first-matmul.
            for c in range(2):
                pg = ps.tile([P, T_CHUNK], F32, name="pg")
                for e in range(E_SUB):
                    nc.tensor.matmul(
                        pg[:], w1q[:, e, :], xv(c, e),
                        start=(e == 0), stop=(e == E_SUB - 1),
                    )
                pu = ps.tile([P, T_CHUNK], F32, name="pu")
                for e in range(E_SUB):
                    nc.tensor.matmul(
                        pu[:], w2q[:, e, :], xv(c, e),
                        start=(e == 0), stop=(e == E_SUB - 1),
                    )
                silu_mul(0, c, pg, pu)

            # slabs 1..63: both chunks back-to-back per weight column
            # block (shared stationary operand on the PE).
            for hb in range(1, N_SLAB):
                w12s = wp.tile([P, E_SUB, 256], CDT, name="w12s")
                nc.sync.dma_start(w12s[:], w12_r[:, hb - 1])

                pgs = [ps.tile([P, T_CHUNK], F32, name="pg") for _ in range(2)]
                for e in range(E_SUB):
                    for c in range(2):
                        nc.tensor.matmul(
                            pgs[c][:], w12s[:, e, 0:P], xv(c, e),
                            start=(e == 0), stop=(e == E_SUB - 1),
                        )
                pus = [ps.tile([P, T_CHUNK], F32, name="pu") for _ in range(2)]
                for e in range(E_SUB):
                    for c in range(2):
                        nc.tensor.matmul(
                            pus[c][:], w12s[:, e, P:256], xv(c, e),
                            start=(e == 0), stop=(e == E_SUB - 1),
                        )
                for c in range(2):
                    silu_mul(hb, c, pgs[c], pus[c])

            # ---------------- phase B: outT = sum_h w3T^T @ hT
            hh = H_SUB // 2

            def out_store(et, c, po):
                e0 = et * P
                ot = op.tile([P, T_CHUNK], CDT, name="ot")
                nc.vector.tensor_copy(ot[:], po[:])
                nc.sync.dma_start(
                    outt[e0 : e0 + P, c * T_CHUNK : (c + 1) * T_CHUNK], ot[:]
                )

            for et in range(E_SUB):
                w3a = w3p.tile([P, hh, P], CDT, name="w3a")
                nc.sync.dma_start(w3a[:], w3t_r[:, et, 0])
                w3b = w3p.tile([P, hh, P], CDT, name="w3b")
                nc.sync.dma_start(w3b[:], w3t_r[:, et, 1])

                def w3v(h):
                    return w3a[:, h, :] if h < hh else w3b[:, h - hh, :]

                if et < E_SUB - 1:
                    pos = [
                        ps.tile([P, T_CHUNK], F32, name="po", bufs=4)
                        for _ in range(2)
                    ]
                    for h in range(H_SUB):
                        for c in range(2):
                            nc.tensor.matmul(
                                pos[c][:], w3v(h),
                                ht[:, h, c * T_CHUNK : (c + 1) * T_CHUNK],
                                start=(h == 0), stop=(h == H_SUB - 1),
                            )
                    for c in range(2):
                        out_store(et, c, pos[c])
                else:
                    # last e-tile: chunk-sequential so chunk 0's copy +
                    # store DMA overlap chunk 1's matmuls (shorter tail)
                    for c in range(2):
                        po = ps.tile([P, T_CHUNK], F32, name="po", bufs=4)
                        for h in range(H_SUB):
                            nc.tensor.matmul(
                                po[:], w3v(h),
                                ht[:, h, c * T_CHUNK : (c + 1) * T_CHUNK],
                                start=(h == 0), stop=(h == H_SUB - 1),
                            )
                        out_store(et, c, po)

    fixed = _split_multi_waits(bass.Bass.to_json_bytes(nc))
    nc.to_json_bytes = lambda: fixed
    return nc


_nc_cache = None


def _get_nc():
    global _nc_cache
    if _nc_cache is None:
        _nc_cache = _build_nc()
    return _nc_cache


def _prep_inputs(x, w1, w2, w3):
    xt = np.ascontiguousarray(
        x.reshape(T_TOTAL, EMB).T.astype(NP_CDT)
    )  # [E, T_total]

    # merged w1|w2 slabs: [p][hb][s][256] with cols 0:128 = w1, 128:256 = w2
    a1 = w1.T.astype(NP_CDT).reshape(E_SUB, P, N_SLAB, P).transpose(1, 2, 0, 3)
    a2 = w2.T.astype(NP_CDT).reshape(E_SUB, P, N_SLAB, P).transpose(1, 2, 0, 3)
    w1h0 = np.ascontiguousarray(a1[:, 0])  # [128, 16, 128]
    w2h0 = np.ascontiguousarray(a2[:, 0])
    w12 = np.ascontiguousarray(
        np.concatenate([a1, a2], axis=3)[:, 1:]
    )  # [128, 63, 16, 256]

    # w3 tiles: [p][et][half][hs][ec]
    w3r = w3.T.astype(NP_CDT).reshape(H_SUB, P, E_SUB, P)
    w3tl = np.ascontiguousarray(
        w3r.transpose(1, 2, 0, 3).reshape(P, E_SUB, 2, H_SUB // 2, P)
    )

    in_maps = []
    for i in range(N_CORES):
        sh = xt[:, i * T_SHARD : (i + 1) * T_SHARD]  # [E, 1024]
        X = sh.reshape(E_SUB, P, T_SHARD)  # [s, p, t]
        xc0 = np.ascontiguousarray(
            X[:, :, :T_CHUNK].reshape(4, 4, P, T_CHUNK).transpose(2, 0, 1, 3)
        )  # [128, 4, 4, 512]
        xc1 = np.ascontiguousarray(
            X[:, :, T_CHUNK:].reshape(4, 4, P, T_CHUNK).transpose(2, 0, 1, 3)
        )
        in_maps.append(
            {
                "xc0": xc0,
                "xc1": xc1,
                "w1h0": w1h0,
                "w2h0": w2h0,
                "w12": w12,
                "w3t": w3tl,
            }
        )
    return in_maps


def kernel(x, w1, w2, w3, scale_x=None, _trace=False):
    x = np.asarray(x, np.float32)
    w1 = np.asarray(w1, np.float32)
    w2 = np.asarray(w2, np.float32)
    w3 = np.asarray(w3, np.float32)

    nc = _get_nc()
    in_maps = _prep_inputs(x, w1, w2, w3)
    res = run_bass_kernel_spmd(nc, in_maps, list(range(N_CORES)), trace=_trace)

    outt = np.concatenate(
        [
            np.asarray(res.results[i]["outt"]).astype(np.float32)
            for i in range(N_CORES)
        ],
        axis=1,
    )  # [E, T_total]
    out = np.ascontiguousarray(outt.T).reshape(4, 2048, EMB).astype(np.float32)
    if _trace:
        kernel.last_results = res
    return out


if __name__ == "__main__":
    rng = np.random.default_rng(0)
    x = rng.standard_normal((4, 2048, EMB), dtype=np.float32)
    w1 = (rng.standard_normal((HID, EMB), dtype=np.float32) * 0.03).astype(
        np.float32
    )
    w2 = (rng.standard_normal((HID, EMB), dtype=np.float32) * 0.03).astype(
        np.float32
    )
    w3 = (rng.standard_normal((EMB, HID), dtype=np.float32) * 0.015).astype(
        np.float32
    )
    out = kernel(x, w1, w2, w3)
    print("out", out.shape, out.dtype, float(np.abs(out).mean()))
